# revision 28
# baseline (speedup 1.0000x reference)
"""EntityBoundaryPredictor Bass kernel for 8 trn2 NeuronCores.

Reference computation (B=4, E=16, T=1024, H=1024, fp32):
    t   = token_embedding @ Wt + bt                       # [B,T,H]
    e   = entity_embedding @ We + be                      # [B,E,H]
    cls = einsum('beth,h->bet', relu(t[:,None]+e[:,:,None]), Wp) + bp
    cls = where(token_mask, cls, -1e4); p = sigmoid(cls)  # returns (cls, p)

Sharding: data-parallel over (b, token-half): core s -> b = s//2,
tokens [th*512,(th+1)*512) with th = s%2.  Weights replicated.

Per-core device plan (h on SBUF partitions throughout):
  - DMA: all inputs pre-arranged on the host so every transfer is
    contiguous per partition (2-8KB lines); split across the sync and
    scalar HWDGE rings with tok + the first weight chunks prioritized so
    the first projection starts ~2us in.  Weights stream in kc-chunks,
    pacing the projection loop.
  - PE: dummy warm-up matmuls during the DMA head (HAM un-throttle),
    then t'(k,t) = Wt^T @ tokT accumulated over 8 h-chunks into PSUM
    (same for e'); ACT folds the bias in during the PSUM->SBUF copy.
  - DVE/ACT/GpSimd: m = relu(t' + e'_scalar) as one fused
    per-partition-scalar op per (e, h-chunk) [128,512] tile, split
    across the three engines (DVE 2x mode ~345ns, GpSimd ~?, ACT ~720ns
    per tile).  A deep m-tile pool lets the producers run ahead while
    the PE is still on DMA-paced projections.
  - PE: cls partial = Wp^T @ m -- an M=32 matvec per (e, h-chunk),
    packed 4-wide into PSUM column groups (partitions 0/32/64/96) for
    column-tile concurrency; 4 entity-group accumulators live across
    the whole h loop.
  - Finalize per entity group as its accumulator completes: ACT evac
    (+bp), DVE copy_predicated onto a NEG-preset tile (mask), ACT
    sigmoid of the masked cls (p inherits the mask; sigmoid(-1e4)=0).
"""

import os

import numpy as np

import bass_rust as _bass_rust
import concourse.bacc as bacc
import concourse.mybir as mybir
from concourse.hw_specs import get_activation_tables
from concourse.tile import TileContext
from concourse.bass_utils import run_bass_kernel_spmd

B, E, T, H = 4, 16, 1024, 1024
P = 128
NCORES = 8
TS = T // 2          # tokens per core
HC = H // P          # h-chunks (contraction)
KC = H // P          # k-chunks (projected feature dim; == h of stage 2)
NEG = -10000.0
NC3 = 3 * KC + 1     # consts columns: btR | beR | wpR | bp

F32 = mybir.dt.float32
F32R = mybir.dt.float32r
BF16 = mybir.dt.bfloat16
F16 = mybir.dt.float16
U8 = mybir.dt.uint8

CFG = {
    "in_dt": os.environ.get("K_IN_DT", "f16"),
    "m_dt": os.environ.get("K_M_DT", "f16"),
    # relu-tile engine split, cycle of 20 tiles: first gp_n on GpSimd,
    # next act_n on ACT, rest on DVE
    "act_frac": float(os.environ.get("K_ACT_FRAC", "0.24")),
    "gp_frac": float(os.environ.get("K_GP_FRAC", "0.0")),
    # m-tile pool depth (backlog while PE finishes DMA-paced projections)
    "m_bufs": int(os.environ.get("K_M_BUFS", "84")),
    # HAM warm-up matmuls (N=512 each) before the first projection
    "warm": int(os.environ.get("K_WARM", "8")),
    # Tile-clock timestamp (ms) before which the bulk weight DMAs stay off
    # the rings, so the head transfers (tok/wt0/we0) aren't slowed by
    # packet-fair interleaving
    "bulk_ms": float(os.environ.get("K_BULK_MS", "0.012")),
    "reps": int(os.environ.get("K_REPS", "1")),
}

_DT = {"f32": F32, "f32r": F32R, "bf16": BF16, "f16": F16}

LAST_RESULTS = None  # BassKernelResults of the most recent run (for test.py)
_BUILT = None        # (cfg_key, nc)


def build(cfg=None):
    cfg = cfg or CFG
    in_dt = _DT[cfg["in_dt"]]
    m_dt = _DT[cfg["m_dt"]]

    nc = bacc.Bacc("TRN2", target_bir_lowering=False, debug=False)

    # All ACT funcs used here (Identity/Relu/Sigmoid) exist in the
    # sigmoid_and_others set; blank the other sets (ids preserved) so a
    # single table load suffices.
    def _one_table_set():
        if not any(
            isinstance(i, mybir.InstActivation)
            for b in nc.main_func.blocks
            for i in b.instructions
        ):
            return
        tables = [
            (n, (f if n == "sigmoid_and_others" else set()))
            for n, f in get_activation_tables(nc.m.arch).items()
        ]
        _bass_rust.insert_act_table_loads(nc, tables)

    nc.insert_act_table_loads = _one_table_set

    tok = nc.declare_dram_parameter("tok", [P, HC, TS], in_dt, isOutput=False)
    wt = nc.declare_dram_parameter("wt", [KC, P, HC, P], in_dt, isOutput=False)
    we = nc.declare_dram_parameter("we", [KC, P, HC, P], in_dt, isOutput=False)
    # ent f16 (256B) | consts f32 (NC3*4B) | mask u8 (TS B), one packed DMA
    SM_ENT = HC * E * 2
    SM_CON = NC3 * 4
    SMB = SM_ENT + SM_CON + TS
    smalls = nc.declare_dram_parameter("smalls", [P, SMB], U8, isOutput=False)

    cls_out = nc.declare_dram_parameter("cls_out", [E, TS], F32, isOutput=True)
    p_out = nc.declare_dram_parameter("p_out", [E, TS], F32, isOutput=True)

    Act = mybir.ActivationFunctionType
    Alu = mybir.AluOpType

    CYC = 20
    gp_n = int(round(cfg["gp_frac"] * CYC))
    act_n = int(round(cfg["act_frac"] * CYC))

    with TileContext(nc) as tc:
        with (
            tc.tile_pool(name="const", bufs=1) as cpool,
            tc.tile_pool(name="mt", bufs=cfg["m_bufs"]) as mpool,
            tc.tile_pool(name="fin", bufs=4) as fpool,
            tc.tile_pool(name="psP", bufs=2, space="PSUM") as psP,
            tc.tile_pool(name="psE", bufs=1, space="PSUM") as psE,
            tc.tile_pool(name="psR", bufs=1, space="PSUM") as psR,
            tc.tile_pool(name="psW", bufs=1, space="PSUM") as psW,
        ):
            rep_ctx = tc.For_i(0, cfg["reps"], 1) if cfg["reps"] > 1 else None
            if rep_ctx is not None:
                rep_ctx.__enter__()

            # ---- input DMAs ------------------------------------------------
            # Both HWDGE rings start moving bytes ~8us into the NEFF and the
            # aggregate HBM rate is ~360 GB/s; order so tok (the projection
            # gate) completes first, then the kc-chunked weight stream paces
            # the projection loop.
            smalls_sb = cpool.tile([P, SMB], U8, tag="smalls")
            tok_sb = cpool.tile([P, HC, TS], in_dt, tag="tok")
            wt_sb = cpool.tile([P, KC, HC, P], in_dt, tag="wt")
            we_sb = cpool.tile([P, KC, HC, P], in_dt, tag="we")

            # warm tile first so the PE warm-up only waits on the memset
            warm = None
            if cfg["warm"] > 0:
                warm = cpool.tile([P, TS], in_dt, tag="warm")
                nc.gpsimd.memset(warm[:, :], 0.0)

            # Queued DMAs on one ring interleave at packet granularity (fair
            # share), so everything queued together finishes late together.
            # Tiny "gate" DMAs that read the head tiles keep the bulk weight
            # stream off the rings (ring FIFO) until the head transfers have
            # completed.
            nc.sync.dma_start(out=smalls_sb[:, :], in_=smalls[:, :])
            nc.sync.dma_start(out=we_sb[:, 0], in_=we[0])
            nc.scalar.dma_start(out=tok_sb[:, 4:8], in_=tok[:, 4:8])
            nc.sync.dma_start(out=tok_sb[:, 0:4], in_=tok[:, 0:4])
            nc.scalar.dma_start(out=wt_sb[:, 0], in_=wt[0])
            with tc.tile_wait_until(cfg["bulk_ms"]):
                nc.sync.dma_start(out=we_sb[:, 1:4], in_=we[1:4].rearrange(
                    "c p h k -> p c h k"))
                nc.scalar.dma_start(out=wt_sb[:, 1:4], in_=wt[1:4].rearrange(
                    "c p h k -> p c h k"))
                nc.sync.dma_start(out=we_sb[:, 4:8], in_=we[4:8].rearrange(
                    "c p h k -> p c h k"))
                nc.scalar.dma_start(out=wt_sb[:, 4:8], in_=wt[4:8].rearrange(
                    "c p h k -> p c h k"))

            ent_sb = smalls_sb[:, 0:SM_ENT].bitcast(F16).rearrange(
                "p (h e) -> p h e", e=E)
            consts_f = smalls_sb[:, SM_ENT:SM_ENT + SM_CON].bitcast(F32)
            mask_sb = smalls_sb[:, SM_ENT + SM_CON:SMB]

            btR = consts_f[:, 0:KC]
            beR = consts_f[:, KC:2 * KC]
            wpR = consts_f[:, 2 * KC:3 * KC]
            bpR = consts_f[:, 3 * KC:3 * KC + 1]

            # combined projection bias: m = relu(traw + (bt+be+eraw)) -- the
            # t' evac stays bias-free and e' absorbs bt+be, so ACT's relu
            # tiles can read t' straight from PSUM with the same scalar.
            btbe = cpool.tile([P, KC], F32, tag="btbe")
            nc.vector.tensor_tensor(
                out=btbe[:, :], in0=btR, in1=beR, op=mybir.AluOpType.add,
            )

            # ---- HAM warm-up: dummy matmuls sized to span the DMA head ----
            if cfg["warm"] > 0:
                wps = psW.tile([P, TS], F32, tag="psW")
                for _ in range(cfg["warm"]):
                    nc.tensor.matmul(
                        wps[:, :], lhsT=warm[:, 0:P], rhs=warm[:, :],
                        start=True, stop=True,
                    )

            # Wp replicated to 32 lhsT columns per h-chunk so the M=32
            # matvec fills a full PSUM column group.
            wp_sb = cpool.tile([P, HC, 32], m_dt, tag="wp")
            for hc in range(HC):
                nc.vector.tensor_copy(
                    out=wp_sb[:, hc, :],
                    in_=wpR[:, hc:hc + 1].broadcast_to([P, 32]),
                )

            # cls staging tiles preset to NEG on the (idle) GpSimd
            clsM_t = []
            for eg in range(E // 4):
                cm = cpool.tile([P, TS], F32, tag=f"clsM{eg}", name=f"clsM{eg}")
                nc.gpsimd.memset(cm[:, :], NEG)
                clsM_t.append(cm)

            # ---- projections (DMA-paced) + relu producers ------------------
            # Engines are in-order, so the relu tiles for h-chunk hc are
            # EMITTED right after kc=hc's evac: DVE/ACT/GpSimd chew on them
            # while the PE waits for the next weight chunk.  The reduce
            # matmuls are deferred past the whole projection loop so they
            # never block a projection in the PE queue; the deep m-pool
            # holds the backlog.
            tp_sb = cpool.tile([P, KC, TS], in_dt, tag="tp")   # traw [k, t]
            ep_sb = cpool.tile([P, KC, E], F32, tag="ep")      # beta [k, e]
            m_tiles = [[None] * E for _ in range(HC)]
            g_tile = 0
            for kc in range(KC):
                # e-projection first: it only needs ent + we[kc], so kc=0
                # starts well before tok lands (and helps warm the PE)
                eps = psE.tile([P, E], F32, tag="psE")
                for hc in range(HC):
                    nc.tensor.matmul(
                        eps[:, :],
                        lhsT=we_sb[:, kc, hc, :],
                        rhs=ent_sb[:, hc, :],
                        start=(hc == 0),
                        stop=(hc == HC - 1),
                    )
                nc.scalar.activation(
                    ep_sb[:, kc, :], eps[:, :], Act.Identity,
                    bias=btbe[:, kc:kc + 1],
                )
                ps = psP.tile([P, TS], F32, tag="psP")
                for hc in range(HC):
                    nc.tensor.matmul(
                        ps[:, :],
                        lhsT=wt_sb[:, kc, hc, :],
                        rhs=tok_sb[:, hc, :],
                        start=(hc == 0),
                        stop=(hc == HC - 1),
                    )
                nc.scalar.activation(
                    tp_sb[:, kc, :], ps[:, :], Act.Identity,
                )
                # relu tiles for hc = kc; ACT's share reads t' straight from
                # the PSUM bank (faster PSUM path + keeps SBUF ports free),
                # DVE's share reads the f16 evac copy.
                hc = kc
                for e in range(E):
                    m = mpool.tile([P, TS], m_dt, tag="m")
                    m_tiles[hc][e] = m
                    lane = g_tile % CYC
                    g_tile += 1
                    if lane < act_n:
                        nc.scalar.activation(
                            m[:, :], tp_sb[:, hc, :], Act.Relu,
                            bias=ep_sb[:, hc, e:e + 1],
                        )
                    else:
                        nc.vector.tensor_scalar(
                            out=m[:, :],
                            in0=tp_sb[:, hc, :],
                            scalar1=ep_sb[:, hc, e:e + 1],
                            scalar2=0.0,
                            op0=Alu.add,
                            op1=Alu.max,
                        )

            # ---- weighted reduction over h (h-outer) + finalize ------------
            # select(mask) straight from PSUM onto the NEG-preset tile, then
            # two independent ACT ops apply +bp (and sigmoid) into shared
            # [P, 4, TS] staging tiles so ONE DMA ships each output.  Masked
            # cls comes out as NEG+bp (3e-6 relative off NEG -- way inside
            # tolerance); masked p is sigmoid(NEG+bp) == 0.0 exactly.
            clsT_big = fpool.tile([P, E // 4, TS], F32, tag="clsT")
            pS_big = fpool.tile([P, E // 4, TS], F32, tag="pS")

            def finalize(eg):
                nc.vector.copy_predicated(
                    clsM_t[eg][:, :], mask_sb[:, :], rps[eg][:, :]
                )
                nc.scalar.activation(
                    clsT_big[:, eg, :], clsM_t[eg][:, :], Act.Identity,
                    bias=bpR[:, 0:1],
                )
                nc.scalar.activation(
                    pS_big[:, eg, :], clsM_t[eg][:, :], Act.Sigmoid,
                    bias=bpR[:, 0:1],
                )

            rps = [psR.tile([P, TS], F32, tag=f"rps{eg}", name=f"rps{eg}")
                   for eg in range(E // 4)]
            for hc in range(HC):
                last_hc = hc == HC - 1
                for e in range(E):
                    eg, j = divmod(e, 4)
                    nc.tensor.matmul(
                        rps[eg][32 * j:32 * j + 32, :],
                        lhsT=wp_sb[:, hc, :],
                        rhs=m_tiles[hc][e][:, :],
                        start=(hc == 0),
                        stop=last_hc,
                        tile_position=(0, 32 * j),
                        # the 4 column groups interleave accumulation in one
                        # bank on disjoint partition ranges; the group
                        # tracker is partition-unaware.
                        skip_group_check=True,
                    )
                    if last_hc and j == 3:
                        finalize(eg)

            # single consolidated output DMAs: src partition j (stride 32)
            # x free (eg, t) matches dst entity e = 4*eg + j
            cls_src = clsT_big[:, :, :].rearrange(
                "(j r) g t -> j r g t", r=32)[:, 0, :, :]
            p_src = pS_big[:, :, :].rearrange(
                "(j r) g t -> j r g t", r=32)[:, 0, :, :]
            cls_dst = cls_out[:, :].rearrange("(g j) t -> j g t", j=4)
            p_dst = p_out[:, :].rearrange("(g j) t -> j g t", j=4)
            nc.sync.dma_start(out=cls_dst, in_=cls_src)
            nc.gpsimd.dma_start(out=p_dst, in_=p_src)

            if rep_ctx is not None:
                rep_ctx.__exit__(None, None, None)

    nc.compile()
    return nc


def _np_dt(name):
    import ml_dtypes

    return {"f32": np.float32, "f32r": np.float32, "bf16": ml_dtypes.bfloat16,
            "f16": np.float16}[name]


def shard_inputs(token_embedding, entity_embedding, token_mask, Wt, bt, We, be,
                 Wp, bp, cfg=None):
    cfg = cfg or CFG
    ind = _np_dt(cfg["in_dt"])
    f32 = np.float32

    # weights: [KC, P, HC, P] with [kc][p, hc, k] = W[hc*P+p, kc*P+k]
    def w_chunks(W):
        w = W.astype(ind, copy=False).reshape(HC, P, KC, P)     # [hc,p,kc,k]
        return np.ascontiguousarray(w.transpose(2, 1, 0, 3))    # [kc,p,hc,k]

    wt_s = w_chunks(Wt)
    we_s = w_chunks(We)
    btR = bt.astype(f32).reshape(KC, P).T
    beR = be.astype(f32).reshape(KC, P).T
    wpR = Wp.astype(f32).reshape(KC, P).T
    bpR = np.broadcast_to(bp.astype(f32).reshape(1, 1), (P, 1))
    consts = np.ascontiguousarray(
        np.concatenate([btR, beR, wpR, bpR], axis=1))
    consts_u8 = consts.view(np.uint8)                     # [P, NC3*4]

    in_maps = []
    for s in range(NCORES):
        b, th = divmod(s, 2)
        tsl = slice(th * TS, (th + 1) * TS)
        # tok: [p, hc, t] = token[b, t0+t, hc*P+p]
        tk = token_embedding[b, tsl, :].astype(ind, copy=False)
        tk = np.ascontiguousarray(
            tk.reshape(TS, HC, P).transpose(2, 1, 0))
        # ent: [p, hc, e] = entity[b, e, hc*P+p]
        en = entity_embedding[b].astype(ind, copy=False)
        en = np.ascontiguousarray(en.reshape(E, HC, P).transpose(2, 1, 0))
        en_u8 = en.reshape(P, -1).view(np.uint8)          # [P, HC*E*2]
        mk = np.broadcast_to(
            token_mask[b, tsl].astype(np.uint8)[None, :], (P, TS))
        sm = np.ascontiguousarray(
            np.concatenate([en_u8, consts_u8, mk], axis=1))
        in_maps.append({
            "tok": tk, "wt": wt_s, "we": we_s, "smalls": sm,
        })
    return in_maps


def kernel(token_embedding, entity_embedding, token_mask, Wt, bt, We, be, Wp, bp):
    global LAST_RESULTS, _BUILT
    cfg_key = tuple(sorted(CFG.items()))
    if _BUILT is None or _BUILT[0] != cfg_key:
        _BUILT = (cfg_key, build(CFG))
    nc = _BUILT[1]

    in_maps = shard_inputs(token_embedding, entity_embedding, token_mask,
                           Wt, bt, We, be, Wp, bp)
    trace = os.environ.get("K_TRACE", "0") == "1"
    res = run_bass_kernel_spmd(nc, in_maps, core_ids=list(range(NCORES)),
                               trace=trace,
                               tmpdir=os.environ.get("K_TRACE_DIR") or None)
    LAST_RESULTS = res

    cls = np.empty((B, E, T), np.float32)
    p = np.empty((B, E, T), np.float32)
    for s in range(NCORES):
        b, th = divmod(s, 2)
        tsl = slice(th * TS, (th + 1) * TS)
        cls[b, :, tsl] = res.results[s]["cls_out"]
        p[b, :, tsl] = res.results[s]["p_out"]
    return cls, p


# revision 30
# speedup vs baseline: 1.0682x; 1.0682x over previous
"""EntityBoundaryPredictor Bass kernel for 8 trn2 NeuronCores.

Reference computation (B=4, E=16, T=1024, H=1024, fp32):
    t   = token_embedding @ Wt + bt                       # [B,T,H]
    e   = entity_embedding @ We + be                      # [B,E,H]
    cls = einsum('beth,h->bet', relu(t[:,None]+e[:,:,None]), Wp) + bp
    cls = where(token_mask, cls, -1e4); p = sigmoid(cls)  # returns (cls, p)

Sharding: data-parallel over (b, token-half): core s -> b = s//2,
tokens [th*512,(th+1)*512) with th = s%2.  Weights replicated.

Per-core device plan (h on SBUF partitions throughout):
  - DMA: all inputs pre-arranged on the host so every transfer is
    contiguous per partition (2-8KB lines); split across the sync and
    scalar HWDGE rings with tok + the first weight chunks prioritized so
    the first projection starts ~2us in.  Weights stream in kc-chunks,
    pacing the projection loop.
  - PE: dummy warm-up matmuls during the DMA head (HAM un-throttle),
    then t'(k,t) = Wt^T @ tokT accumulated over 8 h-chunks into PSUM
    (same for e'); ACT folds the bias in during the PSUM->SBUF copy.
  - DVE/ACT/GpSimd: m = relu(t' + e'_scalar) as one fused
    per-partition-scalar op per (e, h-chunk) [128,512] tile, split
    across the three engines (DVE 2x mode ~345ns, GpSimd ~?, ACT ~720ns
    per tile).  A deep m-tile pool lets the producers run ahead while
    the PE is still on DMA-paced projections.
  - PE: cls partial = Wp^T @ m -- an M=32 matvec per (e, h-chunk),
    packed 4-wide into PSUM column groups (partitions 0/32/64/96) for
    column-tile concurrency; 4 entity-group accumulators live across
    the whole h loop.
  - Finalize per entity group as its accumulator completes: ACT evac
    (+bp), DVE copy_predicated onto a NEG-preset tile (mask), ACT
    sigmoid of the masked cls (p inherits the mask; sigmoid(-1e4)=0).
"""

import os

import numpy as np

import bass_rust as _bass_rust
import concourse.bacc as bacc
import concourse.mybir as mybir
from concourse.hw_specs import get_activation_tables
from concourse.tile import TileContext
from concourse.bass_utils import run_bass_kernel_spmd

B, E, T, H = 4, 16, 1024, 1024
P = 128
NCORES = 8
TS = T // 2          # tokens per core
HC = H // P          # h-chunks (contraction)
KC = H // P          # k-chunks (projected feature dim; == h of stage 2)
NEG = -10000.0
NC3 = 3 * KC + 1     # consts columns: btR | beR | wpR | bp

F32 = mybir.dt.float32
F32R = mybir.dt.float32r
BF16 = mybir.dt.bfloat16
F16 = mybir.dt.float16
U8 = mybir.dt.uint8

CFG = {
    "in_dt": os.environ.get("K_IN_DT", "f16"),
    "m_dt": os.environ.get("K_M_DT", "f16"),
    # relu-tile engine split, cycle of 20 tiles: first gp_n on GpSimd,
    # next act_n on ACT, rest on DVE
    "act_frac": float(os.environ.get("K_ACT_FRAC", "0.24")),
    "gp_frac": float(os.environ.get("K_GP_FRAC", "0.0")),
    # m-tile pool depth (backlog while PE finishes DMA-paced projections)
    "m_bufs": int(os.environ.get("K_M_BUFS", "84")),
    # HAM warm-up matmuls (N=512 each) before the first projection
    "warm": int(os.environ.get("K_WARM", "8")),
    "reps": int(os.environ.get("K_REPS", "1")),
}

_DT = {"f32": F32, "f32r": F32R, "bf16": BF16, "f16": F16}

LAST_RESULTS = None  # BassKernelResults of the most recent run (for test.py)
_BUILT = None        # (cfg_key, nc)


def build(cfg=None):
    cfg = cfg or CFG
    in_dt = _DT[cfg["in_dt"]]
    m_dt = _DT[cfg["m_dt"]]

    nc = bacc.Bacc("TRN2", target_bir_lowering=False, debug=False)

    # All ACT funcs used here (Identity/Relu/Sigmoid) exist in the
    # sigmoid_and_others set; blank the other sets (ids preserved) so a
    # single table load suffices.
    def _one_table_set():
        if not any(
            isinstance(i, mybir.InstActivation)
            for b in nc.main_func.blocks
            for i in b.instructions
        ):
            return
        tables = [
            (n, (f if n == "sigmoid_and_others" else set()))
            for n, f in get_activation_tables(nc.m.arch).items()
        ]
        _bass_rust.insert_act_table_loads(nc, tables)

    nc.insert_act_table_loads = _one_table_set

    tok = nc.declare_dram_parameter("tok", [P, HC, TS], in_dt, isOutput=False)
    wt = nc.declare_dram_parameter("wt", [KC, P, HC, P], in_dt, isOutput=False)
    we = nc.declare_dram_parameter("we", [KC, P, HC, P], in_dt, isOutput=False)
    # ent f16 (256B) | consts f32 (NC3*4B) | mask u8 (TS B), one packed DMA
    SM_ENT = HC * E * 2
    SM_CON = NC3 * 4
    SMB = SM_ENT + SM_CON + TS
    smalls = nc.declare_dram_parameter("smalls", [P, SMB], U8, isOutput=False)

    cls_out = nc.declare_dram_parameter("cls_out", [E, TS], F32, isOutput=True)
    p_out = nc.declare_dram_parameter("p_out", [E, TS], F32, isOutput=True)

    Act = mybir.ActivationFunctionType
    Alu = mybir.AluOpType

    CYC = 20
    gp_n = int(round(cfg["gp_frac"] * CYC))
    act_n = int(round(cfg["act_frac"] * CYC))

    with TileContext(nc) as tc:
        with (
            tc.tile_pool(name="const", bufs=1) as cpool,
            tc.tile_pool(name="mt", bufs=cfg["m_bufs"]) as mpool,
            tc.tile_pool(name="fin", bufs=4) as fpool,
            tc.tile_pool(name="psP", bufs=2, space="PSUM") as psP,
            tc.tile_pool(name="psE", bufs=1, space="PSUM") as psE,
            tc.tile_pool(name="psR", bufs=1, space="PSUM") as psR,
            tc.tile_pool(name="psW", bufs=1, space="PSUM") as psW,
        ):
            rep_ctx = tc.For_i(0, cfg["reps"], 1) if cfg["reps"] > 1 else None
            if rep_ctx is not None:
                rep_ctx.__enter__()

            # ---- input DMAs ------------------------------------------------
            # Both HWDGE rings start moving bytes ~8us into the NEFF and the
            # aggregate HBM rate is ~360 GB/s; order so tok (the projection
            # gate) completes first, then the kc-chunked weight stream paces
            # the projection loop.
            smalls_sb = cpool.tile([P, SMB], U8, tag="smalls")
            tok_sb = cpool.tile([P, HC, TS], in_dt, tag="tok")
            wt_sb = cpool.tile([P, KC, HC, P], in_dt, tag="wt")
            we_sb = cpool.tile([P, KC, HC, P], in_dt, tag="we")

            # warm tile first so the PE warm-up only waits on the memset
            warm = None
            if cfg["warm"] > 0:
                warm = cpool.tile([P, TS], in_dt, tag="warm")
                nc.gpsimd.memset(warm[:, :], 0.0)

            # Queued DMAs on one ring interleave at packet granularity (fair
            # share), so everything queued together finishes late together.
            # Tiny "gate" DMAs that read the head tiles keep the bulk weight
            # stream off the rings (ring FIFO) until the head transfers have
            # completed.
            nc.sync.dma_start(out=smalls_sb[:, :], in_=smalls[:, :])
            nc.sync.dma_start(out=we_sb[:, 0], in_=we[0])
            nc.scalar.dma_start(out=tok_sb[:, 4:8], in_=tok[:, 4:8])
            nc.sync.dma_start(out=tok_sb[:, 0:4], in_=tok[:, 0:4])
            nc.scalar.dma_start(out=wt_sb[:, 0], in_=wt[0])
            # WAR gates: each bulk weight DMA must wait until a tiny GpSimd
            # op has "read" its destination region; that op in turn waits on
            # a head transfer (tok).  Net effect: the bulk stream joins the
            # rings only after the head transfers finish, instead of
            # packet-fair-sharing with them from the start.
            junk_t = cpool.tile([1, 16], in_dt, tag="junk")
            gate_specs = [
                (0, we_sb[0:1, 1, 0, 0:4], tok_sb[0:1, 0, 0:4]),
                (4, we_sb[0:1, 4, 0, 0:4], tok_sb[0:1, 0, 4:8]),
                (8, wt_sb[0:1, 1, 0, 0:4], tok_sb[0:1, 4, 0:4]),
                (12, wt_sb[0:1, 4, 0, 0:4], tok_sb[0:1, 4, 4:8]),
            ]
            for off, war_ap, dep_ap in gate_specs:
                nc.gpsimd.tensor_tensor(
                    out=junk_t[0:1, off:off + 4], in0=war_ap, in1=dep_ap,
                    op=mybir.AluOpType.add,
                )
            nc.sync.dma_start(out=we_sb[:, 1:4], in_=we[1:4].rearrange(
                "c p h k -> p c h k"))
            nc.scalar.dma_start(out=wt_sb[:, 1:4], in_=wt[1:4].rearrange(
                "c p h k -> p c h k"))
            nc.sync.dma_start(out=we_sb[:, 4:8], in_=we[4:8].rearrange(
                "c p h k -> p c h k"))
            nc.scalar.dma_start(out=wt_sb[:, 4:8], in_=wt[4:8].rearrange(
                "c p h k -> p c h k"))

            ent_sb = smalls_sb[:, 0:SM_ENT].bitcast(F16).rearrange(
                "p (h e) -> p h e", e=E)
            consts_f = smalls_sb[:, SM_ENT:SM_ENT + SM_CON].bitcast(F32)
            mask_sb = smalls_sb[:, SM_ENT + SM_CON:SMB]

            btR = consts_f[:, 0:KC]
            beR = consts_f[:, KC:2 * KC]
            wpR = consts_f[:, 2 * KC:3 * KC]
            bpR = consts_f[:, 3 * KC:3 * KC + 1]

            # combined projection bias: m = relu(traw + (bt+be+eraw)) -- the
            # t' evac stays bias-free and e' absorbs bt+be, so ACT's relu
            # tiles can read t' straight from PSUM with the same scalar.
            btbe = cpool.tile([P, KC], F32, tag="btbe")
            nc.vector.tensor_tensor(
                out=btbe[:, :], in0=btR, in1=beR, op=mybir.AluOpType.add,
            )

            # ---- HAM warm-up: dummy matmuls sized to span the DMA head ----
            if cfg["warm"] > 0:
                wps = psW.tile([P, TS], F32, tag="psW")
                for _ in range(cfg["warm"]):
                    nc.tensor.matmul(
                        wps[:, :], lhsT=warm[:, 0:P], rhs=warm[:, :],
                        start=True, stop=True,
                    )

            # Wp replicated to 32 lhsT columns per h-chunk so the M=32
            # matvec fills a full PSUM column group.
            wp_sb = cpool.tile([P, HC, 32], m_dt, tag="wp")
            for hc in range(HC):
                nc.vector.tensor_copy(
                    out=wp_sb[:, hc, :],
                    in_=wpR[:, hc:hc + 1].broadcast_to([P, 32]),
                )

            # cls staging tiles preset to NEG on the (idle) GpSimd
            clsM_t = []
            for eg in range(E // 4):
                cm = cpool.tile([P, TS], F32, tag=f"clsM{eg}", name=f"clsM{eg}")
                nc.gpsimd.memset(cm[:, :], NEG)
                clsM_t.append(cm)

            # ---- projections (DMA-paced) + relu producers ------------------
            # Engines are in-order, so the relu tiles for h-chunk hc are
            # EMITTED right after kc=hc's evac: DVE/ACT/GpSimd chew on them
            # while the PE waits for the next weight chunk.  The reduce
            # matmuls are deferred past the whole projection loop so they
            # never block a projection in the PE queue; the deep m-pool
            # holds the backlog.
            tp_sb = cpool.tile([P, KC, TS], in_dt, tag="tp")   # traw [k, t]
            ep_sb = cpool.tile([P, KC, E], F32, tag="ep")      # beta [k, e]
            m_tiles = [[None] * E for _ in range(HC)]
            g_tile = 0
            for kc in range(KC):
                # e-projection first: it only needs ent + we[kc], so kc=0
                # starts well before tok lands (and helps warm the PE)
                eps = psE.tile([P, E], F32, tag="psE")
                for hc in range(HC):
                    nc.tensor.matmul(
                        eps[:, :],
                        lhsT=we_sb[:, kc, hc, :],
                        rhs=ent_sb[:, hc, :],
                        start=(hc == 0),
                        stop=(hc == HC - 1),
                    )
                nc.scalar.activation(
                    ep_sb[:, kc, :], eps[:, :], Act.Identity,
                    bias=btbe[:, kc:kc + 1],
                )
                ps = psP.tile([P, TS], F32, tag="psP")
                for hc in range(HC):
                    nc.tensor.matmul(
                        ps[:, :],
                        lhsT=wt_sb[:, kc, hc, :],
                        rhs=tok_sb[:, hc, :],
                        start=(hc == 0),
                        stop=(hc == HC - 1),
                    )
                nc.scalar.activation(
                    tp_sb[:, kc, :], ps[:, :], Act.Identity,
                )
                # relu tiles for hc = kc; ACT's share reads t' straight from
                # the PSUM bank (faster PSUM path + keeps SBUF ports free),
                # DVE's share reads the f16 evac copy.
                hc = kc
                for e in range(E):
                    m = mpool.tile([P, TS], m_dt, tag="m")
                    m_tiles[hc][e] = m
                    lane = g_tile % CYC
                    g_tile += 1
                    if lane < act_n:
                        nc.scalar.activation(
                            m[:, :], tp_sb[:, hc, :], Act.Relu,
                            bias=ep_sb[:, hc, e:e + 1],
                        )
                    else:
                        nc.vector.tensor_scalar(
                            out=m[:, :],
                            in0=tp_sb[:, hc, :],
                            scalar1=ep_sb[:, hc, e:e + 1],
                            scalar2=0.0,
                            op0=Alu.add,
                            op1=Alu.max,
                        )

            # ---- weighted reduction over h (h-outer) + finalize ------------
            # select(mask) straight from PSUM onto the NEG-preset tile, then
            # two independent ACT ops apply +bp (and sigmoid) into shared
            # [P, 4, TS] staging tiles so ONE DMA ships each output.  Masked
            # cls comes out as NEG+bp (3e-6 relative off NEG -- way inside
            # tolerance); masked p is sigmoid(NEG+bp) == 0.0 exactly.
            clsT_big = fpool.tile([P, E // 4, TS], F32, tag="clsT")
            pS_big = fpool.tile([P, E // 4, TS], F32, tag="pS")

            def finalize(eg):
                nc.vector.copy_predicated(
                    clsM_t[eg][:, :], mask_sb[:, :], rps[eg][:, :]
                )
                nc.scalar.activation(
                    clsT_big[:, eg, :], clsM_t[eg][:, :], Act.Identity,
                    bias=bpR[:, 0:1],
                )
                nc.scalar.activation(
                    pS_big[:, eg, :], clsM_t[eg][:, :], Act.Sigmoid,
                    bias=bpR[:, 0:1],
                )

            rps = [psR.tile([P, TS], F32, tag=f"rps{eg}", name=f"rps{eg}")
                   for eg in range(E // 4)]
            for hc in range(HC):
                last_hc = hc == HC - 1
                for e in range(E):
                    eg, j = divmod(e, 4)
                    nc.tensor.matmul(
                        rps[eg][32 * j:32 * j + 32, :],
                        lhsT=wp_sb[:, hc, :],
                        rhs=m_tiles[hc][e][:, :],
                        start=(hc == 0),
                        stop=last_hc,
                        tile_position=(0, 32 * j),
                        # the 4 column groups interleave accumulation in one
                        # bank on disjoint partition ranges; the group
                        # tracker is partition-unaware.
                        skip_group_check=True,
                    )
                    if last_hc and j == 3:
                        finalize(eg)

            # single consolidated output DMAs: src partition j (stride 32)
            # x free (eg, t) matches dst entity e = 4*eg + j
            cls_src = clsT_big[:, :, :].rearrange(
                "(j r) g t -> j r g t", r=32)[:, 0, :, :]
            p_src = pS_big[:, :, :].rearrange(
                "(j r) g t -> j r g t", r=32)[:, 0, :, :]
            cls_dst = cls_out[:, :].rearrange("(g j) t -> j g t", j=4)
            p_dst = p_out[:, :].rearrange("(g j) t -> j g t", j=4)
            nc.sync.dma_start(out=cls_dst, in_=cls_src)
            nc.gpsimd.dma_start(out=p_dst, in_=p_src)

            if rep_ctx is not None:
                rep_ctx.__exit__(None, None, None)

    nc.compile()
    return nc


def _np_dt(name):
    import ml_dtypes

    return {"f32": np.float32, "f32r": np.float32, "bf16": ml_dtypes.bfloat16,
            "f16": np.float16}[name]


def shard_inputs(token_embedding, entity_embedding, token_mask, Wt, bt, We, be,
                 Wp, bp, cfg=None):
    cfg = cfg or CFG
    ind = _np_dt(cfg["in_dt"])
    f32 = np.float32

    # weights: [KC, P, HC, P] with [kc][p, hc, k] = W[hc*P+p, kc*P+k]
    def w_chunks(W):
        w = W.astype(ind, copy=False).reshape(HC, P, KC, P)     # [hc,p,kc,k]
        return np.ascontiguousarray(w.transpose(2, 1, 0, 3))    # [kc,p,hc,k]

    wt_s = w_chunks(Wt)
    we_s = w_chunks(We)
    btR = bt.astype(f32).reshape(KC, P).T
    beR = be.astype(f32).reshape(KC, P).T
    wpR = Wp.astype(f32).reshape(KC, P).T
    bpR = np.broadcast_to(bp.astype(f32).reshape(1, 1), (P, 1))
    consts = np.ascontiguousarray(
        np.concatenate([btR, beR, wpR, bpR], axis=1))
    consts_u8 = consts.view(np.uint8)                     # [P, NC3*4]

    in_maps = []
    for s in range(NCORES):
        b, th = divmod(s, 2)
        tsl = slice(th * TS, (th + 1) * TS)
        # tok: [p, hc, t] = token[b, t0+t, hc*P+p]
        tk = token_embedding[b, tsl, :].astype(ind, copy=False)
        tk = np.ascontiguousarray(
            tk.reshape(TS, HC, P).transpose(2, 1, 0))
        # ent: [p, hc, e] = entity[b, e, hc*P+p]
        en = entity_embedding[b].astype(ind, copy=False)
        en = np.ascontiguousarray(en.reshape(E, HC, P).transpose(2, 1, 0))
        en_u8 = en.reshape(P, -1).view(np.uint8)          # [P, HC*E*2]
        mk = np.broadcast_to(
            token_mask[b, tsl].astype(np.uint8)[None, :], (P, TS))
        sm = np.ascontiguousarray(
            np.concatenate([en_u8, consts_u8, mk], axis=1))
        in_maps.append({
            "tok": tk, "wt": wt_s, "we": we_s, "smalls": sm,
        })
    return in_maps


def kernel(token_embedding, entity_embedding, token_mask, Wt, bt, We, be, Wp, bp):
    global LAST_RESULTS, _BUILT
    cfg_key = tuple(sorted(CFG.items()))
    if _BUILT is None or _BUILT[0] != cfg_key:
        _BUILT = (cfg_key, build(CFG))
    nc = _BUILT[1]

    in_maps = shard_inputs(token_embedding, entity_embedding, token_mask,
                           Wt, bt, We, be, Wp, bp)
    trace = os.environ.get("K_TRACE", "0") == "1"
    res = run_bass_kernel_spmd(nc, in_maps, core_ids=list(range(NCORES)),
                               trace=trace,
                               tmpdir=os.environ.get("K_TRACE_DIR") or None)
    LAST_RESULTS = res

    cls = np.empty((B, E, T), np.float32)
    p = np.empty((B, E, T), np.float32)
    for s in range(NCORES):
        b, th = divmod(s, 2)
        tsl = slice(th * TS, (th + 1) * TS)
        cls[b, :, tsl] = res.results[s]["cls_out"]
        p[b, :, tsl] = res.results[s]["p_out"]
    return cls, p


# revision 33
# speedup vs baseline: 1.0700x; 1.0017x over previous
"""EntityBoundaryPredictor Bass kernel for 8 trn2 NeuronCores.

Reference computation (B=4, E=16, T=1024, H=1024, fp32):
    t   = token_embedding @ Wt + bt                       # [B,T,H]
    e   = entity_embedding @ We + be                      # [B,E,H]
    cls = einsum('beth,h->bet', relu(t[:,None]+e[:,:,None]), Wp) + bp
    cls = where(token_mask, cls, -1e4); p = sigmoid(cls)  # returns (cls, p)

Sharding: data-parallel over (b, token-half): core s -> b = s//2,
tokens [th*512,(th+1)*512) with th = s%2.  Weights replicated.

Per-core device plan (h on SBUF partitions throughout):
  - DMA: all inputs pre-arranged on the host so every transfer is
    contiguous per partition (2-8KB lines); split across the sync and
    scalar HWDGE rings with tok + the first weight chunks prioritized so
    the first projection starts ~2us in.  Weights stream in kc-chunks,
    pacing the projection loop.
  - PE: dummy warm-up matmuls during the DMA head (HAM un-throttle),
    then t'(k,t) = Wt^T @ tokT accumulated over 8 h-chunks into PSUM
    (same for e'); ACT folds the bias in during the PSUM->SBUF copy.
  - DVE/ACT/GpSimd: m = relu(t' + e'_scalar) as one fused
    per-partition-scalar op per (e, h-chunk) [128,512] tile, split
    across the three engines (DVE 2x mode ~345ns, GpSimd ~?, ACT ~720ns
    per tile).  A deep m-tile pool lets the producers run ahead while
    the PE is still on DMA-paced projections.
  - PE: cls partial = Wp^T @ m -- an M=32 matvec per (e, h-chunk),
    packed 4-wide into PSUM column groups (partitions 0/32/64/96) for
    column-tile concurrency; 4 entity-group accumulators live across
    the whole h loop.
  - Finalize per entity group as its accumulator completes: ACT evac
    (+bp), DVE copy_predicated onto a NEG-preset tile (mask), ACT
    sigmoid of the masked cls (p inherits the mask; sigmoid(-1e4)=0).
"""

import os

import numpy as np

import bass_rust as _bass_rust
import concourse.bacc as bacc
import concourse.mybir as mybir
from concourse.hw_specs import get_activation_tables
from concourse.tile import TileContext
from concourse.bass_utils import run_bass_kernel_spmd

B, E, T, H = 4, 16, 1024, 1024
P = 128
NCORES = 8
TS = T // 2          # tokens per core
HC = H // P          # h-chunks (contraction)
KC = H // P          # k-chunks (projected feature dim; == h of stage 2)
NEG = -10000.0
NC3 = 3 * KC + 1     # consts columns: btR | beR | wpR | bp

F32 = mybir.dt.float32
F32R = mybir.dt.float32r
BF16 = mybir.dt.bfloat16
F16 = mybir.dt.float16
U8 = mybir.dt.uint8

CFG = {
    "in_dt": os.environ.get("K_IN_DT", "f16"),
    "m_dt": os.environ.get("K_M_DT", "f16"),
    # relu-tile engine split, cycle of 20 tiles: first gp_n on GpSimd,
    # next act_n on ACT, rest on DVE
    "act_frac": float(os.environ.get("K_ACT_FRAC", "0.24")),
    "gp_frac": float(os.environ.get("K_GP_FRAC", "0.0")),
    # m-tile pool depth (backlog while PE finishes DMA-paced projections)
    "m_bufs": int(os.environ.get("K_M_BUFS", "84")),
    # HAM warm-up matmuls (N=512 each) before the first projection
    "warm": int(os.environ.get("K_WARM", "8")),
    "reps": int(os.environ.get("K_REPS", "1")),
}

_DT = {"f32": F32, "f32r": F32R, "bf16": BF16, "f16": F16}

LAST_RESULTS = None  # BassKernelResults of the most recent run (for test.py)
_BUILT = None        # (cfg_key, nc)


def build(cfg=None):
    cfg = cfg or CFG
    in_dt = _DT[cfg["in_dt"]]
    m_dt = _DT[cfg["m_dt"]]

    nc = bacc.Bacc("TRN2", target_bir_lowering=False, debug=False)

    # All ACT funcs used here (Identity/Relu/Sigmoid) exist in the
    # sigmoid_and_others set; blank the other sets (ids preserved) so a
    # single table load suffices.
    def _one_table_set():
        if not any(
            isinstance(i, mybir.InstActivation)
            for b in nc.main_func.blocks
            for i in b.instructions
        ):
            return
        tables = [
            (n, (f if n == "sigmoid_and_others" else set()))
            for n, f in get_activation_tables(nc.m.arch).items()
        ]
        _bass_rust.insert_act_table_loads(nc, tables)

    nc.insert_act_table_loads = _one_table_set

    tok = nc.declare_dram_parameter("tok", [P, HC, TS], in_dt, isOutput=False)
    wt = nc.declare_dram_parameter("wt", [KC, P, HC, P], in_dt, isOutput=False)
    we = nc.declare_dram_parameter("we", [KC, P, HC, P], in_dt, isOutput=False)
    # ent f16 (256B) | consts f32 (NC3*4B) | mask u8 (TS B), one packed DMA
    SM_ENT = HC * E * 2
    SM_CON = NC3 * 4
    SMB = SM_ENT + SM_CON + TS
    smalls = nc.declare_dram_parameter("smalls", [P, SMB], U8, isOutput=False)

    cls_out = nc.declare_dram_parameter("cls_out", [E, TS], F32, isOutput=True)
    p_out = nc.declare_dram_parameter("p_out", [E, TS], F32, isOutput=True)

    Act = mybir.ActivationFunctionType
    Alu = mybir.AluOpType

    CYC = 20
    gp_n = int(round(cfg["gp_frac"] * CYC))
    act_n = int(round(cfg["act_frac"] * CYC))

    with TileContext(nc) as tc:
        with (
            tc.tile_pool(name="const", bufs=1) as cpool,
            tc.tile_pool(name="mt", bufs=cfg["m_bufs"]) as mpool,
            tc.tile_pool(name="fin", bufs=4) as fpool,
            tc.tile_pool(name="psP", bufs=2, space="PSUM") as psP,
            tc.tile_pool(name="psE", bufs=1, space="PSUM") as psE,
            tc.tile_pool(name="psR", bufs=1, space="PSUM") as psR,
            tc.tile_pool(name="psW", bufs=1, space="PSUM") as psW,
        ):
            rep_ctx = tc.For_i(0, cfg["reps"], 1) if cfg["reps"] > 1 else None
            if rep_ctx is not None:
                rep_ctx.__enter__()

            # ---- input DMAs ------------------------------------------------
            # Both HWDGE rings start moving bytes ~8us into the NEFF and the
            # aggregate HBM rate is ~360 GB/s; order so tok (the projection
            # gate) completes first, then the kc-chunked weight stream paces
            # the projection loop.
            smalls_sb = cpool.tile([P, SMB], U8, tag="smalls")
            tok_sb = cpool.tile([P, HC, TS], in_dt, tag="tok")
            wt_sb = cpool.tile([P, KC, HC, P], in_dt, tag="wt")
            we_sb = cpool.tile([P, KC, HC, P], in_dt, tag="we")

            # warm tile first so the PE warm-up only waits on the memset
            warm = None
            if cfg["warm"] > 0:
                warm = cpool.tile([P, TS], in_dt, tag="warm")
                nc.gpsimd.memset(warm[:, :], 0.0)

            # Queued DMAs on one ring interleave at packet granularity (fair
            # share), so everything queued together finishes late together.
            # Tiny "gate" DMAs that read the head tiles keep the bulk weight
            # stream off the rings (ring FIFO) until the head transfers have
            # completed.
            nc.scalar.dma_start(out=smalls_sb[:, :], in_=smalls[:, :])
            nc.sync.dma_start(out=tok_sb[:, :, :], in_=tok[:, :, :])
            nc.scalar.dma_start(out=wt_sb[:, 0], in_=wt[0])
            nc.scalar.dma_start(out=we_sb[:, 0], in_=we[0])
            # WAR gates: each bulk weight DMA must wait until a tiny GpSimd
            # op has "read" its destination region; that op in turn waits on
            # a head transfer (tok).  Net effect: the bulk stream joins the
            # rings only after the head transfers finish, instead of
            # packet-fair-sharing with them from the start.
            junk_t = cpool.tile([1, 16], in_dt, tag="junk")
            gate_specs = [
                (0, we_sb[0:1, 1, 0, 0:4], tok_sb[0:1, 0, 0:4]),
                (4, we_sb[0:1, 4, 0, 0:4], tok_sb[0:1, 4, 0:4]),
                (8, wt_sb[0:1, 1, 0, 0:4], we_sb[0:1, 0, 0, 0:4]),
                (12, wt_sb[0:1, 4, 0, 0:4], we_sb[0:1, 0, 0, 4:8]),
            ]
            for off, war_ap, dep_ap in gate_specs:
                nc.gpsimd.tensor_tensor(
                    out=junk_t[0:1, off:off + 4], in0=war_ap, in1=dep_ap,
                    op=mybir.AluOpType.add,
                )
            nc.sync.dma_start(out=we_sb[:, 1:4], in_=we[1:4].rearrange(
                "c p h k -> p c h k"))
            nc.scalar.dma_start(out=wt_sb[:, 1:4], in_=wt[1:4].rearrange(
                "c p h k -> p c h k"))
            nc.sync.dma_start(out=we_sb[:, 4:8], in_=we[4:8].rearrange(
                "c p h k -> p c h k"))
            nc.scalar.dma_start(out=wt_sb[:, 4:8], in_=wt[4:8].rearrange(
                "c p h k -> p c h k"))

            ent_sb = smalls_sb[:, 0:SM_ENT].bitcast(F16).rearrange(
                "p (h e) -> p h e", e=E)
            consts_f = smalls_sb[:, SM_ENT:SM_ENT + SM_CON].bitcast(F32)
            mask_sb = smalls_sb[:, SM_ENT + SM_CON:SMB]

            btR = consts_f[:, 0:KC]
            beR = consts_f[:, KC:2 * KC]
            wpR = consts_f[:, 2 * KC:3 * KC]
            bpR = consts_f[:, 3 * KC:3 * KC + 1]

            # combined projection bias: m = relu(traw + (bt+be+eraw)) -- the
            # t' evac stays bias-free and e' absorbs bt+be, so ACT's relu
            # tiles can read t' straight from PSUM with the same scalar.
            btbe = cpool.tile([P, KC], F32, tag="btbe")
            nc.vector.tensor_tensor(
                out=btbe[:, :], in0=btR, in1=beR, op=mybir.AluOpType.add,
            )

            # ---- HAM warm-up: dummy matmuls sized to span the DMA head ----
            if cfg["warm"] > 0:
                wps = psW.tile([P, TS], F32, tag="psW")
                for _ in range(cfg["warm"]):
                    nc.tensor.matmul(
                        wps[:, :], lhsT=warm[:, 0:P], rhs=warm[:, :],
                        start=True, stop=True,
                    )

            # Wp replicated to 32 lhsT columns per h-chunk so the M=32
            # matvec fills a full PSUM column group.
            wp_sb = cpool.tile([P, HC, 32], m_dt, tag="wp")
            for hc in range(HC):
                nc.vector.tensor_copy(
                    out=wp_sb[:, hc, :],
                    in_=wpR[:, hc:hc + 1].broadcast_to([P, 32]),
                )

            # cls staging tiles preset to NEG on the (idle) GpSimd
            clsM_t = []
            for eg in range(E // 4):
                cm = cpool.tile([P, TS], F32, tag=f"clsM{eg}", name=f"clsM{eg}")
                nc.gpsimd.memset(cm[:, :], NEG)
                clsM_t.append(cm)

            # ---- projections (DMA-paced) + relu producers ------------------
            # Engines are in-order, so the relu tiles for h-chunk hc are
            # EMITTED right after kc=hc's evac: DVE/ACT/GpSimd chew on them
            # while the PE waits for the next weight chunk.  The reduce
            # matmuls are deferred past the whole projection loop so they
            # never block a projection in the PE queue; the deep m-pool
            # holds the backlog.
            tp_sb = cpool.tile([P, KC, TS], in_dt, tag="tp")   # traw [k, t]
            ep_sb = cpool.tile([P, KC, E], F32, tag="ep")      # beta [k, e]
            m_tiles = [[None] * E for _ in range(HC)]
            g_tile = 0

            def relu_tiles(hc):
                nonlocal g_tile
                for e in range(E):
                    m = mpool.tile([P, TS], m_dt, tag="m")
                    m_tiles[hc][e] = m
                    lane = g_tile % CYC
                    g_tile += 1
                    if lane < act_n:
                        nc.scalar.activation(
                            m[:, :], tp_sb[:, hc, :], Act.Relu,
                            bias=ep_sb[:, hc, e:e + 1],
                        )
                    else:
                        nc.vector.tensor_scalar(
                            out=m[:, :],
                            in0=tp_sb[:, hc, :],
                            scalar1=ep_sb[:, hc, e:e + 1],
                            scalar2=0.0,
                            op0=Alu.add,
                            op1=Alu.max,
                        )

            for kc in range(KC):
                # e-projection first: it only needs ent + we[kc], so kc=0
                # starts well before tok lands (and helps warm the PE)
                eps = psE.tile([P, E], F32, tag="psE")
                for hc in range(HC):
                    nc.tensor.matmul(
                        eps[:, :],
                        lhsT=we_sb[:, kc, hc, :],
                        rhs=ent_sb[:, hc, :],
                        start=(hc == 0),
                        stop=(hc == HC - 1),
                    )
                nc.scalar.activation(
                    ep_sb[:, kc, :], eps[:, :], Act.Identity,
                    bias=btbe[:, kc:kc + 1],
                )
                ps = psP.tile([P, TS], F32, tag="psP")
                for hc in range(HC):
                    nc.tensor.matmul(
                        ps[:, :],
                        lhsT=wt_sb[:, kc, hc, :],
                        rhs=tok_sb[:, hc, :],
                        start=(hc == 0),
                        stop=(hc == HC - 1),
                    )
                nc.scalar.activation(
                    tp_sb[:, kc, :], ps[:, :], Act.Identity,
                )
                # relu tiles for the PREVIOUS chunk: keeps the next chunk's
                # evacs ahead of relu work in the in-order ACT queue, so the
                # DVE never stalls waiting for an evac stuck behind relus.
                if kc >= 1:
                    relu_tiles(kc - 1)
            relu_tiles(KC - 1)

            # ---- weighted reduction over h (h-outer) + finalize ------------
            # select(mask) straight from PSUM onto the NEG-preset tile, then
            # two independent ACT ops apply +bp (and sigmoid) into shared
            # [P, 4, TS] staging tiles so ONE DMA ships each output.  Masked
            # cls comes out as NEG+bp (3e-6 relative off NEG -- way inside
            # tolerance); masked p is sigmoid(NEG+bp) == 0.0 exactly.
            clsT_big = fpool.tile([P, E // 4, TS], F32, tag="clsT")
            pS_big = fpool.tile([P, E // 4, TS], F32, tag="pS")

            def finalize(eg):
                nc.vector.copy_predicated(
                    clsM_t[eg][:, :], mask_sb[:, :], rps[eg][:, :]
                )
                nc.scalar.activation(
                    clsT_big[:, eg, :], clsM_t[eg][:, :], Act.Identity,
                    bias=bpR[:, 0:1],
                )
                nc.scalar.activation(
                    pS_big[:, eg, :], clsM_t[eg][:, :], Act.Sigmoid,
                    bias=bpR[:, 0:1],
                )

            rps = [psR.tile([P, TS], F32, tag=f"rps{eg}", name=f"rps{eg}")
                   for eg in range(E // 4)]
            for hc in range(HC):
                last_hc = hc == HC - 1
                for e in range(E):
                    eg, j = divmod(e, 4)
                    nc.tensor.matmul(
                        rps[eg][32 * j:32 * j + 32, :],
                        lhsT=wp_sb[:, hc, :],
                        rhs=m_tiles[hc][e][:, :],
                        start=(hc == 0),
                        stop=last_hc,
                        tile_position=(0, 32 * j),
                        # the 4 column groups interleave accumulation in one
                        # bank on disjoint partition ranges; the group
                        # tracker is partition-unaware.
                        skip_group_check=True,
                    )
                    if last_hc and j == 3:
                        finalize(eg)

            # single consolidated output DMAs: src partition j (stride 32)
            # x free (eg, t) matches dst entity e = 4*eg + j
            cls_src = clsT_big[:, :, :].rearrange(
                "(j r) g t -> j r g t", r=32)[:, 0, :, :]
            p_src = pS_big[:, :, :].rearrange(
                "(j r) g t -> j r g t", r=32)[:, 0, :, :]
            cls_dst = cls_out[:, :].rearrange("(g j) t -> j g t", j=4)
            p_dst = p_out[:, :].rearrange("(g j) t -> j g t", j=4)
            nc.sync.dma_start(out=cls_dst, in_=cls_src)
            nc.gpsimd.dma_start(out=p_dst, in_=p_src)

            if rep_ctx is not None:
                rep_ctx.__exit__(None, None, None)

    nc.compile()
    return nc


def _np_dt(name):
    import ml_dtypes

    return {"f32": np.float32, "f32r": np.float32, "bf16": ml_dtypes.bfloat16,
            "f16": np.float16}[name]


def shard_inputs(token_embedding, entity_embedding, token_mask, Wt, bt, We, be,
                 Wp, bp, cfg=None):
    cfg = cfg or CFG
    ind = _np_dt(cfg["in_dt"])
    f32 = np.float32

    # weights: [KC, P, HC, P] with [kc][p, hc, k] = W[hc*P+p, kc*P+k]
    def w_chunks(W):
        w = W.astype(ind, copy=False).reshape(HC, P, KC, P)     # [hc,p,kc,k]
        return np.ascontiguousarray(w.transpose(2, 1, 0, 3))    # [kc,p,hc,k]

    wt_s = w_chunks(Wt)
    we_s = w_chunks(We)
    btR = bt.astype(f32).reshape(KC, P).T
    beR = be.astype(f32).reshape(KC, P).T
    wpR = Wp.astype(f32).reshape(KC, P).T
    bpR = np.broadcast_to(bp.astype(f32).reshape(1, 1), (P, 1))
    consts = np.ascontiguousarray(
        np.concatenate([btR, beR, wpR, bpR], axis=1))
    consts_u8 = consts.view(np.uint8)                     # [P, NC3*4]

    in_maps = []
    for s in range(NCORES):
        b, th = divmod(s, 2)
        tsl = slice(th * TS, (th + 1) * TS)
        # tok: [p, hc, t] = token[b, t0+t, hc*P+p]
        tk = token_embedding[b, tsl, :].astype(ind, copy=False)
        tk = np.ascontiguousarray(
            tk.reshape(TS, HC, P).transpose(2, 1, 0))
        # ent: [p, hc, e] = entity[b, e, hc*P+p]
        en = entity_embedding[b].astype(ind, copy=False)
        en = np.ascontiguousarray(en.reshape(E, HC, P).transpose(2, 1, 0))
        en_u8 = en.reshape(P, -1).view(np.uint8)          # [P, HC*E*2]
        mk = np.broadcast_to(
            token_mask[b, tsl].astype(np.uint8)[None, :], (P, TS))
        sm = np.ascontiguousarray(
            np.concatenate([en_u8, consts_u8, mk], axis=1))
        in_maps.append({
            "tok": tk, "wt": wt_s, "we": we_s, "smalls": sm,
        })
    return in_maps


def kernel(token_embedding, entity_embedding, token_mask, Wt, bt, We, be, Wp, bp):
    global LAST_RESULTS, _BUILT
    cfg_key = tuple(sorted(CFG.items()))
    if _BUILT is None or _BUILT[0] != cfg_key:
        _BUILT = (cfg_key, build(CFG))
    nc = _BUILT[1]

    in_maps = shard_inputs(token_embedding, entity_embedding, token_mask,
                           Wt, bt, We, be, Wp, bp)
    trace = os.environ.get("K_TRACE", "0") == "1"
    res = run_bass_kernel_spmd(nc, in_maps, core_ids=list(range(NCORES)),
                               trace=trace,
                               tmpdir=os.environ.get("K_TRACE_DIR") or None)
    LAST_RESULTS = res

    cls = np.empty((B, E, T), np.float32)
    p = np.empty((B, E, T), np.float32)
    for s in range(NCORES):
        b, th = divmod(s, 2)
        tsl = slice(th * TS, (th + 1) * TS)
        cls[b, :, tsl] = res.results[s]["cls_out"]
        p[b, :, tsl] = res.results[s]["p_out"]
    return cls, p


# revision 35
# speedup vs baseline: 1.1019x; 1.0299x over previous
"""EntityBoundaryPredictor Bass kernel for 8 trn2 NeuronCores.

Reference computation (B=4, E=16, T=1024, H=1024, fp32):
    t   = token_embedding @ Wt + bt                       # [B,T,H]
    e   = entity_embedding @ We + be                      # [B,E,H]
    cls = einsum('beth,h->bet', relu(t[:,None]+e[:,:,None]), Wp) + bp
    cls = where(token_mask, cls, -1e4); p = sigmoid(cls)  # returns (cls, p)

Sharding: data-parallel over (b, token-half): core s -> b = s//2,
tokens [th*512,(th+1)*512) with th = s%2.  Weights replicated.

Per-core device plan (h on SBUF partitions throughout):
  - DMA: all inputs pre-arranged on the host so every transfer is
    contiguous per partition (2-8KB lines); split across the sync and
    scalar HWDGE rings with tok + the first weight chunks prioritized so
    the first projection starts ~2us in.  Weights stream in kc-chunks,
    pacing the projection loop.
  - PE: dummy warm-up matmuls during the DMA head (HAM un-throttle),
    then t'(k,t) = Wt^T @ tokT accumulated over 8 h-chunks into PSUM
    (same for e'); ACT folds the bias in during the PSUM->SBUF copy.
  - DVE/ACT/GpSimd: m = relu(t' + e'_scalar) as one fused
    per-partition-scalar op per (e, h-chunk) [128,512] tile, split
    across the three engines (DVE 2x mode ~345ns, GpSimd ~?, ACT ~720ns
    per tile).  A deep m-tile pool lets the producers run ahead while
    the PE is still on DMA-paced projections.
  - PE: cls partial = Wp^T @ m -- an M=32 matvec per (e, h-chunk),
    packed 4-wide into PSUM column groups (partitions 0/32/64/96) for
    column-tile concurrency; 4 entity-group accumulators live across
    the whole h loop.
  - Finalize per entity group as its accumulator completes: ACT evac
    (+bp), DVE copy_predicated onto a NEG-preset tile (mask), ACT
    sigmoid of the masked cls (p inherits the mask; sigmoid(-1e4)=0).
"""

import os

import numpy as np

import bass_rust as _bass_rust
import concourse.bacc as bacc
import concourse.mybir as mybir
from concourse.hw_specs import get_activation_tables
from concourse.tile import TileContext
from concourse.bass_utils import run_bass_kernel_spmd

B, E, T, H = 4, 16, 1024, 1024
P = 128
NCORES = 8
TS = T // 2          # tokens per core
HC = H // P          # h-chunks (contraction)
KC = H // P          # k-chunks (projected feature dim; == h of stage 2)
NEG = -10000.0
NC3 = 3 * KC + 1     # consts columns: btR | beR | wpR | bp

F32 = mybir.dt.float32
F32R = mybir.dt.float32r
BF16 = mybir.dt.bfloat16
F16 = mybir.dt.float16
U8 = mybir.dt.uint8

CFG = {
    "in_dt": os.environ.get("K_IN_DT", "f16"),
    "m_dt": os.environ.get("K_M_DT", "f16"),
    # relu-tile engine split, cycle of 20 tiles: first gp_n on GpSimd,
    # next act_n on ACT, rest on DVE
    "act_frac": float(os.environ.get("K_ACT_FRAC", "0.24")),
    "gp_frac": float(os.environ.get("K_GP_FRAC", "0.0")),
    # m-tile pool depth (backlog while PE finishes DMA-paced projections)
    "m_bufs": int(os.environ.get("K_M_BUFS", "84")),
    # HAM warm-up matmuls (N=512 each) before the first projection
    "warm": int(os.environ.get("K_WARM", "8")),
    "reps": int(os.environ.get("K_REPS", "1")),
}

_DT = {"f32": F32, "f32r": F32R, "bf16": BF16, "f16": F16}

LAST_RESULTS = None  # BassKernelResults of the most recent run (for test.py)
_BUILT = None        # (cfg_key, nc)


def build(cfg=None):
    cfg = cfg or CFG
    in_dt = _DT[cfg["in_dt"]]
    m_dt = _DT[cfg["m_dt"]]

    nc = bacc.Bacc("TRN2", target_bir_lowering=False, debug=False)

    # All ACT funcs used here (Identity/Relu/Sigmoid) exist in the
    # sigmoid_and_others set; blank the other sets (ids preserved) so a
    # single table load suffices.
    def _one_table_set():
        if not any(
            isinstance(i, mybir.InstActivation)
            for b in nc.main_func.blocks
            for i in b.instructions
        ):
            return
        tables = [
            (n, (f if n == "sigmoid_and_others" else set()))
            for n, f in get_activation_tables(nc.m.arch).items()
        ]
        _bass_rust.insert_act_table_loads(nc, tables)

    nc.insert_act_table_loads = _one_table_set

    tok = nc.declare_dram_parameter("tok", [P, HC, TS], in_dt, isOutput=False)
    wt = nc.declare_dram_parameter("wt", [KC, P, HC, P], in_dt, isOutput=False)
    we = nc.declare_dram_parameter("we", [KC, P, HC, P], in_dt, isOutput=False)
    # ent f16 (256B) | consts f32 (NC3*4B) | mask u8 (TS B), one packed DMA
    SM_ENT = HC * E * 2
    SM_CON = NC3 * 4
    SMB = SM_ENT + SM_CON + TS
    smalls = nc.declare_dram_parameter("smalls", [P, SMB], U8, isOutput=False)

    cls_out = nc.declare_dram_parameter("cls_out", [E, TS], F32, isOutput=True)
    p_out = nc.declare_dram_parameter("p_out", [E, TS], F32, isOutput=True)

    Act = mybir.ActivationFunctionType
    Alu = mybir.AluOpType

    CYC = 20
    gp_n = int(round(cfg["gp_frac"] * CYC))
    act_n = int(round(cfg["act_frac"] * CYC))

    with TileContext(nc) as tc:
        with (
            tc.tile_pool(name="const", bufs=1) as cpool,
            tc.tile_pool(name="mt", bufs=cfg["m_bufs"]) as mpool,
            tc.tile_pool(name="fin", bufs=4) as fpool,
            tc.tile_pool(name="psP", bufs=2, space="PSUM") as psP,
            tc.tile_pool(name="psE", bufs=1, space="PSUM") as psE,
            tc.tile_pool(name="psR", bufs=1, space="PSUM") as psR,
            tc.tile_pool(name="psW", bufs=1, space="PSUM") as psW,
        ):
            rep_ctx = tc.For_i(0, cfg["reps"], 1) if cfg["reps"] > 1 else None
            if rep_ctx is not None:
                rep_ctx.__enter__()

            # ---- input DMAs ------------------------------------------------
            # Both HWDGE rings start moving bytes ~8us into the NEFF and the
            # aggregate HBM rate is ~360 GB/s; order so tok (the projection
            # gate) completes first, then the kc-chunked weight stream paces
            # the projection loop.
            smalls_sb = cpool.tile([P, SMB], U8, tag="smalls")
            tok_sb = cpool.tile([P, HC, TS], in_dt, tag="tok")
            wt_sb = cpool.tile([P, KC, HC, P], in_dt, tag="wt")
            we_sb = cpool.tile([P, KC, HC, P], in_dt, tag="we")

            # warm tile first so the PE warm-up only waits on the memset
            warm = None
            if cfg["warm"] > 0:
                warm = cpool.tile([P, TS], in_dt, tag="warm")
                nc.gpsimd.memset(warm[:, :], 0.0)

            # Queued DMAs on one ring interleave at packet granularity (fair
            # share), so everything queued together finishes late together.
            # Tiny "gate" DMAs that read the head tiles keep the bulk weight
            # stream off the rings (ring FIFO) until the head transfers have
            # completed.
            nc.scalar.dma_start(out=smalls_sb[:, :], in_=smalls[:, :])
            nc.sync.dma_start(out=tok_sb[:, :, :], in_=tok[:, :, :])
            nc.scalar.dma_start(out=wt_sb[:, 0], in_=wt[0])
            nc.scalar.dma_start(out=we_sb[:, 0], in_=we[0])
            # WAR gates serialize each ring's bulk weight stream into a
            # chain of links, each starting only after the previous link's
            # bytes landed: a tiny GpSimd op "reads" the next link's
            # destination region (WAR -> that DMA waits for it) and takes
            # the previous link's data as its other input (RAW).  Queued
            # DMAs on a ring otherwise interleave packet-fair and everything
            # finishes late together.
            junk_t = cpool.tile([1, 32], in_dt, tag="junk")
            LINKS = [(1, 3), (3, 5), (5, 8)]
            gi = 0

            def gate(war_ap, dep_ap):
                nonlocal gi
                nc.gpsimd.tensor_tensor(
                    out=junk_t[0:1, gi:gi + 4], in0=war_ap, in1=dep_ap,
                    op=mybir.AluOpType.add,
                )
                gi += 4

            for lo, hi in LINKS:
                dep = (tok_sb[0:1, 7, 0:4] if lo == 1
                       else we_sb[0:1, lo - 1, 0, 0:4])
                gate(we_sb[0:1, lo, 0, 0:4], dep)
                nc.sync.dma_start(out=we_sb[:, lo:hi], in_=we[lo:hi].rearrange(
                    "c p h k -> p c h k"))
            for lo, hi in LINKS:
                dep = (we_sb[0:1, 0, 0, 0:4] if lo == 1
                       else wt_sb[0:1, lo - 1, 0, 0:4])
                gate(wt_sb[0:1, lo, 0, 0:4], dep)
                nc.scalar.dma_start(out=wt_sb[:, lo:hi], in_=wt[lo:hi].rearrange(
                    "c p h k -> p c h k"))

            ent_sb = smalls_sb[:, 0:SM_ENT].bitcast(F16).rearrange(
                "p (h e) -> p h e", e=E)
            consts_f = smalls_sb[:, SM_ENT:SM_ENT + SM_CON].bitcast(F32)
            mask_sb = smalls_sb[:, SM_ENT + SM_CON:SMB]

            btR = consts_f[:, 0:KC]
            beR = consts_f[:, KC:2 * KC]
            wpR = consts_f[:, 2 * KC:3 * KC]
            bpR = consts_f[:, 3 * KC:3 * KC + 1]

            # combined projection bias: m = relu(traw + (bt+be+eraw)) -- the
            # t' evac stays bias-free and e' absorbs bt+be, so ACT's relu
            # tiles can read t' straight from PSUM with the same scalar.
            btbe = cpool.tile([P, KC], F32, tag="btbe")
            nc.vector.tensor_tensor(
                out=btbe[:, :], in0=btR, in1=beR, op=mybir.AluOpType.add,
            )

            # ---- HAM warm-up: dummy matmuls sized to span the DMA head ----
            if cfg["warm"] > 0:
                wps = psW.tile([P, TS], F32, tag="psW")
                for _ in range(cfg["warm"]):
                    nc.tensor.matmul(
                        wps[:, :], lhsT=warm[:, 0:P], rhs=warm[:, :],
                        start=True, stop=True,
                    )

            # Wp replicated to 32 lhsT columns per h-chunk so the M=32
            # matvec fills a full PSUM column group.
            wp_sb = cpool.tile([P, HC, 32], m_dt, tag="wp")
            for hc in range(HC):
                nc.vector.tensor_copy(
                    out=wp_sb[:, hc, :],
                    in_=wpR[:, hc:hc + 1].broadcast_to([P, 32]),
                )

            # cls staging tiles preset to NEG on the (idle) GpSimd
            clsM_t = []
            for eg in range(E // 4):
                cm = cpool.tile([P, TS], F32, tag=f"clsM{eg}", name=f"clsM{eg}")
                nc.gpsimd.memset(cm[:, :], NEG)
                clsM_t.append(cm)

            # ---- projections (DMA-paced) + relu producers ------------------
            # Engines are in-order, so the relu tiles for h-chunk hc are
            # EMITTED right after kc=hc's evac: DVE/ACT/GpSimd chew on them
            # while the PE waits for the next weight chunk.  The reduce
            # matmuls are deferred past the whole projection loop so they
            # never block a projection in the PE queue; the deep m-pool
            # holds the backlog.
            tp_sb = cpool.tile([P, KC, TS], in_dt, tag="tp")   # traw [k, t]
            ep_sb = cpool.tile([P, KC, E], F32, tag="ep")      # beta [k, e]
            m_tiles = [[None] * E for _ in range(HC)]
            g_tile = 0

            def relu_tiles(hc):
                nonlocal g_tile
                for e in range(E):
                    m = mpool.tile([P, TS], m_dt, tag="m")
                    m_tiles[hc][e] = m
                    lane = g_tile % CYC
                    g_tile += 1
                    if lane < act_n:
                        nc.scalar.activation(
                            m[:, :], tp_sb[:, hc, :], Act.Relu,
                            bias=ep_sb[:, hc, e:e + 1],
                        )
                    else:
                        nc.vector.tensor_scalar(
                            out=m[:, :],
                            in0=tp_sb[:, hc, :],
                            scalar1=ep_sb[:, hc, e:e + 1],
                            scalar2=0.0,
                            op0=Alu.add,
                            op1=Alu.max,
                        )

            for kc in range(KC):
                ps = psP.tile([P, TS], F32, tag="psP")
                for hc in range(HC):
                    nc.tensor.matmul(
                        ps[:, :],
                        lhsT=wt_sb[:, kc, hc, :],
                        rhs=tok_sb[:, hc, :],
                        start=(hc == 0),
                        stop=(hc == HC - 1),
                    )
                nc.scalar.activation(
                    tp_sb[:, kc, :], ps[:, :], Act.Identity,
                )
                eps = psE.tile([P, E], F32, tag="psE")
                for hc in range(HC):
                    nc.tensor.matmul(
                        eps[:, :],
                        lhsT=we_sb[:, kc, hc, :],
                        rhs=ent_sb[:, hc, :],
                        start=(hc == 0),
                        stop=(hc == HC - 1),
                    )
                nc.scalar.activation(
                    ep_sb[:, kc, :], eps[:, :], Act.Identity,
                    bias=btbe[:, kc:kc + 1],
                )
                # relu tiles for the PREVIOUS chunk: keeps the next chunk's
                # evacs ahead of relu work in the in-order ACT queue, so the
                # DVE never stalls waiting for an evac stuck behind relus.
                if kc >= 1:
                    relu_tiles(kc - 1)
            relu_tiles(KC - 1)

            # ---- weighted reduction over h (h-outer) + finalize ------------
            # select(mask) straight from PSUM onto the NEG-preset tile, then
            # two independent ACT ops apply +bp (and sigmoid) into shared
            # [P, 4, TS] staging tiles so ONE DMA ships each output.  Masked
            # cls comes out as NEG+bp (3e-6 relative off NEG -- way inside
            # tolerance); masked p is sigmoid(NEG+bp) == 0.0 exactly.
            clsT_big = fpool.tile([P, E // 4, TS], F32, tag="clsT")
            pS_big = fpool.tile([P, E // 4, TS], F32, tag="pS")

            def finalize(eg):
                nc.vector.copy_predicated(
                    clsM_t[eg][:, :], mask_sb[:, :], rps[eg][:, :]
                )
                nc.scalar.activation(
                    clsT_big[:, eg, :], clsM_t[eg][:, :], Act.Identity,
                    bias=bpR[:, 0:1],
                )
                nc.scalar.activation(
                    pS_big[:, eg, :], clsM_t[eg][:, :], Act.Sigmoid,
                    bias=bpR[:, 0:1],
                )

            rps = [psR.tile([P, TS], F32, tag=f"rps{eg}", name=f"rps{eg}")
                   for eg in range(E // 4)]
            for hc in range(HC):
                last_hc = hc == HC - 1
                for e in range(E):
                    eg, j = divmod(e, 4)
                    nc.tensor.matmul(
                        rps[eg][32 * j:32 * j + 32, :],
                        lhsT=wp_sb[:, hc, :],
                        rhs=m_tiles[hc][e][:, :],
                        start=(hc == 0),
                        stop=last_hc,
                        tile_position=(0, 32 * j),
                        # the 4 column groups interleave accumulation in one
                        # bank on disjoint partition ranges; the group
                        # tracker is partition-unaware.
                        skip_group_check=True,
                    )
                    if last_hc and j == 3:
                        finalize(eg)

            # single consolidated output DMAs: src partition j (stride 32)
            # x free (eg, t) matches dst entity e = 4*eg + j
            cls_src = clsT_big[:, :, :].rearrange(
                "(j r) g t -> j r g t", r=32)[:, 0, :, :]
            p_src = pS_big[:, :, :].rearrange(
                "(j r) g t -> j r g t", r=32)[:, 0, :, :]
            cls_dst = cls_out[:, :].rearrange("(g j) t -> j g t", j=4)
            p_dst = p_out[:, :].rearrange("(g j) t -> j g t", j=4)
            nc.sync.dma_start(out=cls_dst, in_=cls_src)
            nc.gpsimd.dma_start(out=p_dst, in_=p_src)

            if rep_ctx is not None:
                rep_ctx.__exit__(None, None, None)

    nc.compile()
    return nc


def _np_dt(name):
    import ml_dtypes

    return {"f32": np.float32, "f32r": np.float32, "bf16": ml_dtypes.bfloat16,
            "f16": np.float16}[name]


def shard_inputs(token_embedding, entity_embedding, token_mask, Wt, bt, We, be,
                 Wp, bp, cfg=None):
    cfg = cfg or CFG
    ind = _np_dt(cfg["in_dt"])
    f32 = np.float32

    # weights: [KC, P, HC, P] with [kc][p, hc, k] = W[hc*P+p, kc*P+k]
    def w_chunks(W):
        w = W.astype(ind, copy=False).reshape(HC, P, KC, P)     # [hc,p,kc,k]
        return np.ascontiguousarray(w.transpose(2, 1, 0, 3))    # [kc,p,hc,k]

    wt_s = w_chunks(Wt)
    we_s = w_chunks(We)
    btR = bt.astype(f32).reshape(KC, P).T
    beR = be.astype(f32).reshape(KC, P).T
    wpR = Wp.astype(f32).reshape(KC, P).T
    bpR = np.broadcast_to(bp.astype(f32).reshape(1, 1), (P, 1))
    consts = np.ascontiguousarray(
        np.concatenate([btR, beR, wpR, bpR], axis=1))
    consts_u8 = consts.view(np.uint8)                     # [P, NC3*4]

    in_maps = []
    for s in range(NCORES):
        b, th = divmod(s, 2)
        tsl = slice(th * TS, (th + 1) * TS)
        # tok: [p, hc, t] = token[b, t0+t, hc*P+p]
        tk = token_embedding[b, tsl, :].astype(ind, copy=False)
        tk = np.ascontiguousarray(
            tk.reshape(TS, HC, P).transpose(2, 1, 0))
        # ent: [p, hc, e] = entity[b, e, hc*P+p]
        en = entity_embedding[b].astype(ind, copy=False)
        en = np.ascontiguousarray(en.reshape(E, HC, P).transpose(2, 1, 0))
        en_u8 = en.reshape(P, -1).view(np.uint8)          # [P, HC*E*2]
        mk = np.broadcast_to(
            token_mask[b, tsl].astype(np.uint8)[None, :], (P, TS))
        sm = np.ascontiguousarray(
            np.concatenate([en_u8, consts_u8, mk], axis=1))
        in_maps.append({
            "tok": tk, "wt": wt_s, "we": we_s, "smalls": sm,
        })
    return in_maps


def kernel(token_embedding, entity_embedding, token_mask, Wt, bt, We, be, Wp, bp):
    global LAST_RESULTS, _BUILT
    cfg_key = tuple(sorted(CFG.items()))
    if _BUILT is None or _BUILT[0] != cfg_key:
        _BUILT = (cfg_key, build(CFG))
    nc = _BUILT[1]

    in_maps = shard_inputs(token_embedding, entity_embedding, token_mask,
                           Wt, bt, We, be, Wp, bp)
    trace = os.environ.get("K_TRACE", "0") == "1"
    res = run_bass_kernel_spmd(nc, in_maps, core_ids=list(range(NCORES)),
                               trace=trace,
                               tmpdir=os.environ.get("K_TRACE_DIR") or None)
    LAST_RESULTS = res

    cls = np.empty((B, E, T), np.float32)
    p = np.empty((B, E, T), np.float32)
    for s in range(NCORES):
        b, th = divmod(s, 2)
        tsl = slice(th * TS, (th + 1) * TS)
        cls[b, :, tsl] = res.results[s]["cls_out"]
        p[b, :, tsl] = res.results[s]["p_out"]
    return cls, p
